# revision 2
# baseline (speedup 1.0000x reference)
"""Trainium2 Bass kernel for nn_FeatureLossOursBMSE.

Model: s = conv1x1(preds_S) -> masked by checkerboard -> conv3x3 -> relu ->
conv3x3 = new_fea (t). Then pairwise Gram q[i,j] = <p_i, t_j> over D=C*H*W,
logits = -0.5*sq/64, ce = mean_i(logsumexp_j - diag), loss = ce*16*2e-5.
||p_i||^2 cancels exactly in (logsumexp_j logits[i,:] - logits[i,i]), so only
q[i,j] and ||t_j||^2 are needed:
  L[i,j] = q[i,j]/64 - ||t_j||^2/128;  ce_i = logsumexp_j L[i,j] - L[i,i].

Sharding: 8 cores, horizontal slab of 8 image rows per core, all 8 images.
Each core computes conv stack on its slab (with halo rows computed locally),
partial Gram q and ||t||^2 over its slab (D-sharded contraction), then one
72-float AllReduce and a replicated 8x8 softmax-CE tail.

Conv implementation: fp32r matmuls (full-rate on TRN2 for free>=256),
width-padded feature tiles [128, rows, 66] with zeroed border columns, taps
shift the rhs AP by kw in {0,1,2}; PSUM accumulates 18 matmuls per output
block. Checkerboard mask and align-conv bias are folded host-side: the mask
commutes with the 1x1 conv, and the bias rides an extra contraction row
whose input channel is the (row-validity-masked) mask itself.
"""

import numpy as np
from contextlib import ExitStack

import concourse.bass as bass
import concourse.mybir as mybir
import concourse.tile as tile
from concourse import bacc
from concourse.bass_utils import run_bass_kernel_spmd

F32 = mybir.dt.float32
F32R = mybir.dt.float32r
AF = mybir.ActivationFunctionType
ALU = mybir.AluOpType

N_CORES = 8
N, CS, CT, H, W = 8, 128, 256, 64, 64
RS = H // N_CORES  # slab rows per core = 8
NOISE_VAR = 64.0
ALPHA_MGD = 2e-05

LAST_RESULTS = None
_NC_CACHE = {}


def _build():
    nc = bacc.Bacc("TRN2", target_bir_lowering=False, debug=False, num_devices=N_CORES)
    xs_in = nc.dram_tensor("xs", [N, 128, 12, 64], F32R, kind="ExternalInput").ap()
    xsm_in = nc.dram_tensor("xsm", [N, 1, 12, 64], F32R, kind="ExternalInput").ap()
    wa_in = nc.dram_tensor("wa", [128, 2, 128], F32R, kind="ExternalInput").ap()
    wam_in = nc.dram_tensor("wam", [1, 2, 128], F32R, kind="ExternalInput").ap()
    w1_in = nc.dram_tensor("w1", [128, 9, 2, 2, 128], F32R, kind="ExternalInput").ap()
    w2_in = nc.dram_tensor("w2", [128, 9, 2, 2, 128], F32R, kind="ExternalInput").ap()
    b1_in = nc.dram_tensor("b1", [2, 128, 1], F32, kind="ExternalInput").ap()
    b2_in = nc.dram_tensor("b2", [2, 128, 1], F32, kind="ExternalInput").ap()
    rm_in = nc.dram_tensor("rm", [128, 2], F32, kind="ExternalInput").ap()
    p_in = nc.dram_tensor("p", [128, N, 2, RS, 64], F32, kind="ExternalInput").ap()
    zz_in = nc.dram_tensor("zz", [128, 12, 2], F32R, kind="ExternalInput").ap()
    eye_in = nc.dram_tensor("eye8", [8, 8], F32, kind="ExternalInput").ap()
    loss_out = nc.dram_tensor("loss", [1, 1], F32, kind="ExternalOutput").ap()

    with tile.TileContext(nc) as tc:
        with ExitStack() as ctx:
            wpool = ctx.enter_context(tc.tile_pool(name="weights", bufs=1))
            xpool = ctx.enter_context(tc.tile_pool(name="xs", bufs=3))
            fpool = ctx.enter_context(tc.tile_pool(name="feat", bufs=1))
            ppool = ctx.enter_context(tc.tile_pool(name="p", bufs=1))
            spool = ctx.enter_context(tc.tile_pool(name="scratch", bufs=2))
            cpool = ctx.enter_context(tc.tile_pool(name="ce", bufs=1))
            psum = ctx.enter_context(tc.tile_pool(name="psum", bufs=6, space="PSUM"))
            cps = ctx.enter_context(tc.tile_pool(name="ceps", bufs=2, space="PSUM"))
            dram = ctx.enter_context(tc.tile_pool(name="dram", bufs=1, space="DRAM"))

            # ---- weights / constants ----
            wa_t = wpool.tile([128, 2, 128], F32R)
            wam_t = wpool.tile([1, 2, 128], F32R)
            w1_t = wpool.tile([128, 9, 2, 2, 128], F32R)
            w2_t = wpool.tile([128, 9, 2, 2, 128], F32R)
            b1_t = wpool.tile([128, 2], F32)
            b2_t = wpool.tile([128, 2], F32)
            rm_t = wpool.tile([128, 2], F32)
            eye_t = wpool.tile([8, 8], F32)
            nc.sync.dma_start(out=wa_t[:], in_=wa_in)
            nc.sync.dma_start(out=wam_t[:], in_=wam_in)
            nc.sync.dma_start(out=w1_t[:], in_=w1_in)
            nc.sync.dma_start(out=w2_t[:], in_=w2_in)
            for oi in range(2):
                nc.sync.dma_start(out=b1_t[:, oi : oi + 1], in_=b1_in[oi])
                nc.sync.dma_start(out=b2_t[:, oi : oi + 1], in_=b2_in[oi])
            nc.sync.dma_start(out=rm_t[:], in_=rm_in)
            nc.sync.dma_start(out=eye_t[:], in_=eye_in)

            p_t = ppool.tile([128, N, 2, RS, 64], F32)
            nc.sync.dma_start(out=p_t[:], in_=p_in)

            ones = cpool.tile([128, 1], F32)
            nc.vector.memset(ones[:], 1.0)
            ones_row = cpool.tile([1, 8], F32)
            nc.vector.memset(ones_row[:], 1.0)

            # ---- persistent feature tiles (border cols pre-zeroed) ----
            masked = [
                [
                    fpool.tile([128, 12, 66], F32R, name=f"mk{par}_{oi}", tag=f"mk{par}_{oi}")
                    for oi in range(2)
                ]
                for par in range(2)
            ]
            relu1 = [
                [
                    fpool.tile([128, 10, 66], F32R, name=f"r1{par}_{oi}", tag=f"r1{par}_{oi}")
                    for oi in range(2)
                ]
                for par in range(2)
            ]
            t_tiles = [
                [
                    fpool.tile([128, RS, 64], F32, name=f"t{n}_{oi}", tag=f"t{n}_{oi}")
                    for oi in range(2)
                ]
                for n in range(N)
            ]
            for par in range(2):
                for oi in range(2):
                    nc.sync.dma_start(out=masked[par][oi][:, :, 0:1], in_=zz_in[:, 0:12, 0:1])
                    nc.sync.dma_start(out=masked[par][oi][:, :, 65:66], in_=zz_in[:, 0:12, 1:2])
                    nc.sync.dma_start(out=relu1[par][oi][:, :, 0:1], in_=zz_in[:, 0:10, 0:1])
                    nc.sync.dma_start(out=relu1[par][oi][:, :, 65:66], in_=zz_in[:, 0:10, 1:2])

            # gram accumulators: acc[:, (i*8+n)*2+ci], tnorm[:, n*2+ci]
            acc = cpool.tile([128, 128], F32)
            tnorm = cpool.tile([128, 16], F32)

            # ---- per-image stages ----
            def conv1x1(n):
                xs_t = xpool.tile([128, 12, 64], F32R, name="xs_t", tag="xs_t")
                xsm_t = xpool.tile([1, 12, 64], F32R, name="xsm_t", tag="xsm_t")
                nc.sync.dma_start(out=xs_t[:], in_=xs_in[n])
                nc.sync.dma_start(out=xsm_t[:], in_=xsm_in[n])
                mk = masked[n % 2]
                for oi in range(2):
                    for r0, R in ((0, 8), (8, 4)):
                        ps = psum.tile([128, 512], F32, name="ps_c1", tag="ps")
                        ps3 = ps[:, 0 : R * 64].rearrange("p (r c) -> p r c", r=R)
                        nc.tensor.matmul(
                            ps3, wa_t[:, oi, :], xs_t[:, r0 : r0 + R, :],
                            start=True, stop=False,
                        )
                        nc.tensor.matmul(
                            ps3, wam_t[:, oi, :], xsm_t[:, r0 : r0 + R, :],
                            start=False, stop=True,
                        )
                        nc.scalar.copy(mk[oi][:, r0 : r0 + R, 1:65], ps3)

            def conv3x3(w_t, src, b_t, n, rows_out, fb, dst_fn, func):
                # dst_fn(oi) -> (out_ap rows x 64). src: 2-chunk padded tiles.
                for oi in range(2):
                    for r0, R in fb:
                        ps = psum.tile([128, 512], F32, name="ps_g", tag="ps")
                        ps3 = ps[:, 0 : R * 64].rearrange("p (r c) -> p r c", r=R)
                        k = 0
                        for t in range(9):
                            kh, kw = t // 3, t % 3
                            for ci in range(2):
                                nc.tensor.matmul(
                                    ps3,
                                    w_t[:, t, ci, oi, :],
                                    src[ci][:, r0 + kh : r0 + kh + R, kw : kw + 64],
                                    start=(k == 0),
                                    stop=(k == 17),
                                )
                                k += 1
                        nc.scalar.activation(
                            dst_fn(oi, r0, R), ps3, func, bias=b_t[:, oi : oi + 1],
                        )

            def gen1(n):
                mk, r1 = masked[n % 2], relu1[n % 2]
                conv3x3(
                    w1_t, mk, b1_t, n, 10, ((0, 5), (5, 5)),
                    lambda oi, r0, R: r1[oi][:, r0 : r0 + R, 1:65],
                    AF.Relu,
                )
                # zero out-of-image halo rows (rows 0 and 9) via rowmask
                for oi in range(2):
                    hal = r1[oi][:, 0:10:9, :]
                    nc.vector.tensor_tensor(
                        out=hal, in0=hal,
                        in1=rm_t[:, :, None].to_broadcast([128, 2, 66]),
                        op=ALU.mult,
                    )

            def gen2(n):
                r1 = relu1[n % 2]
                conv3x3(
                    w2_t, r1, b2_t, n, RS, ((0, 4), (4, 4)),
                    lambda oi, r0, R: t_tiles[n][oi][:, r0 : r0 + R, :],
                    AF.Identity,
                )

            def gram(n):
                for ci in range(2):
                    tt = t_tiles[n][ci]
                    sq = spool.tile([128, 512], F32, name="sq_s", tag="sq_s")
                    nc.scalar.activation(
                        sq[:].rearrange("p (r c) -> p r c", r=RS), tt[:], AF.Square,
                        accum_out=tnorm[:, n * 2 + ci : n * 2 + ci + 1],
                    )
                    for i in range(N):
                        gs = spool.tile([128, 512], F32, name="gr_s", tag="gr_s")
                        col = (i * 8 + n) * 2 + ci
                        nc.vector.affine_mul_reduce(
                            out=gs[:].rearrange("p (r c) -> p r c", r=RS),
                            accum_out=acc[:, col : col + 1],
                            in0=p_t[:, i, ci],
                            in1=tt[:],
                            scale=1.0,
                            bias=0.0,
                        )

            conv1x1(0)
            conv1x1(1)
            for n in range(N):
                gen1(n)
                gen2(n)
                if n + 2 < N:
                    conv1x1(n + 2)
                gram(n)

            # ---- CE tail (replicated on every core) ----
            acc64 = cpool.tile([128, 64], F32)
            accv = acc[:].rearrange("p (q c) -> p q c", c=2)
            nc.vector.tensor_tensor(out=acc64[:], in0=accv[:, :, 0], in1=accv[:, :, 1], op=ALU.add)
            tn8 = cpool.tile([128, 8], F32)
            tnv = tnorm[:].rearrange("p (q c) -> p q c", c=2)
            nc.vector.tensor_tensor(out=tn8[:], in0=tnv[:, :, 0], in1=tnv[:, :, 1], op=ALU.add)

            part = cps.tile([1, 72], F32, name="part", tag="ceps")
            nc.tensor.matmul(part[:, 0:64], ones[:], acc64[:], start=True, stop=False)
            nc.tensor.matmul(part[:, 64:72], ones[:], tn8[:], start=False, stop=True)
            part_sb = cpool.tile([1, 72], F32)
            nc.scalar.copy(part_sb[:], part[:])

            cc_in = dram.tile([1, 72], F32)
            cc_out = dram.tile([1, 72], F32)
            nc.sync.dma_start(out=cc_in[:], in_=part_sb[:])
            nc.gpsimd.collective_compute(
                "AllReduce", ALU.add, replica_groups=[list(range(N_CORES))],
                ins=[cc_in[:].opt()], outs=[cc_out[:].opt()],
            )

            q_sb = cpool.tile([8, 8], F32)
            qd = cpool.tile([8, 1], F32)
            tn_row = cpool.tile([1, 8], F32)
            tn_p = cpool.tile([8, 1], F32)
            cc_flat = cc_out[:].rearrange("a b -> (a b)")
            nc.sync.dma_start(
                out=q_sb[:], in_=cc_out[:, 0:64].rearrange("a (i j) -> (a i) j", i=8)
            )
            nc.sync.dma_start(out=qd[:], in_=cc_flat[0:64][::9].rearrange("(i j) -> i j", j=1))
            nc.sync.dma_start(out=tn_row[:], in_=cc_out[:, 64:72])
            nc.sync.dma_start(out=tn_p[:], in_=cc_flat[64:72].rearrange("(i j) -> i j", j=1))

            # L[i,j] = q/64 - tn[j]/128 via PE broadcast
            q_s = cpool.tile([8, 8], F32)
            nc.scalar.mul(q_s[:], q_sb[:], 1.0 / NOISE_VAR)
            tn_neg = cpool.tile([1, 8], F32)
            nc.scalar.mul(tn_neg[:], tn_row[:], -1.0 / (2.0 * NOISE_VAR))
            L_ps = cps.tile([8, 8], F32, name="L_ps", tag="ceps")
            nc.tensor.matmul(L_ps[:], eye_t[:], q_s[:], start=True, stop=False)
            nc.tensor.matmul(L_ps[:], ones_row[:], tn_neg[:], start=False, stop=True)
            L = cpool.tile([8, 8], F32)
            nc.vector.tensor_copy(L[:], L_ps[:])

            m = cpool.tile([8, 1], F32)
            nc.vector.reduce_max(m[:], L[:], axis=mybir.AxisListType.X)
            negm = cpool.tile([8, 1], F32)
            nc.scalar.mul(negm[:], m[:], -1.0)
            e = cpool.tile([8, 8], F32)
            nc.scalar.activation(e[:], L[:], AF.Exp, bias=negm[:, 0:1], scale=1.0)
            s = cpool.tile([8, 1], F32)
            nc.vector.reduce_sum(s[:], e[:], axis=mybir.AxisListType.X)
            ln_s = cpool.tile([8, 1], F32)
            nc.scalar.activation(ln_s[:], s[:], AF.Ln)

            ce = cpool.tile([8, 1], F32)
            nc.vector.tensor_add(ce[:], m[:], ln_s[:])
            t1 = cpool.tile([8, 1], F32)
            nc.scalar.mul(t1[:], qd[:], 1.0 / NOISE_VAR)
            nc.vector.tensor_sub(ce[:], ce[:], t1[:])
            t2 = cpool.tile([8, 1], F32)
            nc.scalar.mul(t2[:], tn_p[:], 1.0 / (2.0 * NOISE_VAR))
            nc.vector.tensor_add(ce[:], ce[:], t2[:])

            lp = cps.tile([1, 1], F32, name="lp", tag="ceps")
            nc.tensor.matmul(lp[:], ce[:], ones[0:8, :], start=True, stop=True)
            l_sb = cpool.tile([1, 1], F32)
            # loss = sum_i ce_i / N * (2*NOISE_VAR/N) * ALPHA
            nc.scalar.mul(l_sb[:], lp[:], (2.0 * NOISE_VAR / (N * N)) * ALPHA_MGD)
            nc.sync.dma_start(out=loss_out, in_=l_sb[:])
    nc.compile()
    return nc


def _prep_inputs(preds_S, preds_T, W_align, b_align, W_gen1, b_gen1, W_gen2, b_gen2):
    f32 = np.float32
    mat = ((np.arange(H)[:, None] + np.arange(W)[None, :]) % 2).astype(f32)

    wa = np.ascontiguousarray(W_align[:, :, 0, 0].T.reshape(128, 2, 128), f32)
    wam = np.ascontiguousarray(b_align.reshape(1, 2, 128), f32)

    def pack_w(Wg):
        w = Wg.reshape(2, 128, 2, 128, 3, 3)  # [oi, o, ci, i, kh, kw]
        w = w.transpose(3, 4, 5, 2, 0, 1)  # [i, kh, kw, ci, oi, o]
        return np.ascontiguousarray(w.reshape(128, 9, 2, 2, 128), f32)

    w1 = pack_w(np.asarray(W_gen1, f32))
    w2 = pack_w(np.asarray(W_gen2, f32))
    b1 = np.ascontiguousarray(b_gen1.reshape(2, 128, 1), f32)
    b2 = np.ascontiguousarray(b_gen2.reshape(2, 128, 1), f32)
    eye8 = np.eye(8, dtype=f32)
    zz = np.zeros((128, 12, 2), f32)

    in_maps = []
    for c in range(N_CORES):
        rows = np.arange(8 * c - 2, 8 * c + 10)
        valid = (rows >= 0) & (rows < H)
        vr = rows[valid]
        xs = np.zeros((N, 128, 12, 64), f32)
        xs[:, :, valid] = preds_S[:, :, vr, :] * mat[vr][None, None]
        xsm = np.zeros((N, 1, 12, 64), f32)
        xsm[:, 0, valid] = mat[vr]
        rm = np.broadcast_to(
            np.array([1.0 if c > 0 else 0.0, 1.0 if c < 7 else 0.0], f32), (128, 2)
        ).copy()
        slab = preds_T[:, :, 8 * c : 8 * c + RS, :].reshape(N, 2, 128, RS, 64)
        p = np.ascontiguousarray(slab.transpose(2, 0, 1, 3, 4), f32)
        in_maps.append(
            {
                "xs": xs, "xsm": xsm, "wa": wa, "wam": wam, "w1": w1, "w2": w2,
                "b1": b1, "b2": b2, "rm": rm, "p": p, "zz": zz, "eye8": eye8,
            }
        )
    return in_maps


def kernel(preds_S, preds_T, W_align, b_align, W_gen1, b_gen1, W_gen2, b_gen2):
    global LAST_RESULTS
    preds_S = np.asarray(preds_S, np.float32)
    preds_T = np.asarray(preds_T, np.float32)
    in_maps = _prep_inputs(
        preds_S, preds_T,
        np.asarray(W_align, np.float32), np.asarray(b_align, np.float32),
        np.asarray(W_gen1, np.float32), np.asarray(b_gen1, np.float32),
        np.asarray(W_gen2, np.float32), np.asarray(b_gen2, np.float32),
    )
    if "nc" not in _NC_CACHE:
        _NC_CACHE["nc"] = _build()
    res = run_bass_kernel_spmd(_NC_CACHE["nc"], in_maps, core_ids=list(range(N_CORES)))
    LAST_RESULTS = res
    return np.float32(res.results[0]["loss"][0, 0])


# revision 4
# speedup vs baseline: 1.3331x; 1.3331x over previous
"""Trainium2 Bass kernel for nn_FeatureLossOursBMSE.

Model: s = conv1x1(preds_S) -> masked by checkerboard -> conv3x3 -> relu ->
conv3x3 = new_fea (t). Then pairwise Gram q[i,j] = <p_i, t_j> over D=C*H*W,
logits = -0.5*sq/64, ce = mean_i(logsumexp_j - diag), loss = ce*16*2e-5.
||p_i||^2 cancels exactly in (logsumexp_j logits[i,:] - logits[i,i]), so only
q[i,j] and ||t_j||^2 are needed:
  L[i,j] = q[i,j]/64 - ||t_j||^2/128;  ce_i = logsumexp_j L[i,j] - L[i,i].

Sharding: 8 cores, horizontal slab of 8 image rows per core, all 8 images.
Each core computes conv stack on its slab (with halo rows computed locally),
partial Gram q and ||t||^2 over its slab (D-sharded contraction), then one
72-float AllReduce and a replicated 8x8 softmax-CE tail.

Conv implementation: fp32r matmuls (full-rate on TRN2 for free>=256),
width-padded feature tiles [128, rows, 66] with zeroed border columns, taps
shift the rhs AP by kw in {0,1,2}; PSUM accumulates 18 matmuls per output
block. Checkerboard mask and align-conv bias are folded host-side: the mask
commutes with the 1x1 conv, and the bias rides an extra contraction row
whose input channel is the (row-validity-masked) mask itself.
"""

import numpy as np
from contextlib import ExitStack

import concourse.bass as bass
import concourse.mybir as mybir
import concourse.tile as tile
from concourse import bacc
from concourse.bass_utils import run_bass_kernel_spmd

F32 = mybir.dt.float32
F32R = mybir.dt.float32r
AF = mybir.ActivationFunctionType
ALU = mybir.AluOpType

N_CORES = 8
N, CS, CT, H, W = 8, 128, 256, 64, 64
RS = H // N_CORES  # slab rows per core = 8
NOISE_VAR = 64.0
ALPHA_MGD = 2e-05

LAST_RESULTS = None
_NC_CACHE = {}


def _build():
    nc = bacc.Bacc("TRN2", target_bir_lowering=False, debug=False, num_devices=N_CORES)
    xs_in = nc.dram_tensor("xs", [N, 128, 12, 64], F32R, kind="ExternalInput").ap()
    xsm_in = nc.dram_tensor("xsm", [N, 1, 12, 64], F32R, kind="ExternalInput").ap()
    wa_in = nc.dram_tensor("wa", [128, 2, 128], F32R, kind="ExternalInput").ap()
    wam_in = nc.dram_tensor("wam", [1, 2, 128], F32R, kind="ExternalInput").ap()
    w1_in = nc.dram_tensor("w1", [128, 9, 2, 2, 128], F32R, kind="ExternalInput").ap()
    w2_in = nc.dram_tensor("w2", [128, 9, 2, 2, 128], F32R, kind="ExternalInput").ap()
    b1_in = nc.dram_tensor("b1", [2, 128, 1], F32, kind="ExternalInput").ap()
    b2_in = nc.dram_tensor("b2", [2, 128, 1], F32, kind="ExternalInput").ap()
    rm_in = nc.dram_tensor("rm", [128, 2], F32, kind="ExternalInput").ap()
    p_in = nc.dram_tensor("p", [128, N, 2, RS, 64], F32, kind="ExternalInput").ap()
    zz_in = nc.dram_tensor("zz", [128, 12, 2], F32R, kind="ExternalInput").ap()
    eye_in = nc.dram_tensor("eye8", [8, 8], F32, kind="ExternalInput").ap()
    loss_out = nc.dram_tensor("loss", [1, 1], F32, kind="ExternalOutput").ap()

    with tile.TileContext(nc) as tc:
        with ExitStack() as ctx:
            wpool = ctx.enter_context(tc.tile_pool(name="weights", bufs=1))
            xpool = ctx.enter_context(tc.tile_pool(name="xs", bufs=3))
            fpool = ctx.enter_context(tc.tile_pool(name="feat", bufs=1))
            ppool = ctx.enter_context(tc.tile_pool(name="p", bufs=1))
            spool = ctx.enter_context(tc.tile_pool(name="scratch", bufs=2))
            cpool = ctx.enter_context(tc.tile_pool(name="ce", bufs=1))
            psum = ctx.enter_context(tc.tile_pool(name="psum", bufs=6, space="PSUM"))
            cps = ctx.enter_context(tc.tile_pool(name="ceps", bufs=2, space="PSUM"))
            dram = ctx.enter_context(tc.tile_pool(name="dram", bufs=1, space="DRAM"))

            # ---- weights / constants ----
            wa_t = wpool.tile([128, 2, 128], F32R)
            wam_t = wpool.tile([1, 2, 128], F32R)
            w1_t = wpool.tile([128, 9, 2, 2, 128], F32R)
            w2_t = wpool.tile([128, 9, 2, 2, 128], F32R)
            b1_t = wpool.tile([128, 2], F32)
            b2_t = wpool.tile([128, 2], F32)
            rm_t = wpool.tile([128, 2], F32)
            eye_t = wpool.tile([8, 8], F32)
            nc.sync.dma_start(out=wa_t[:], in_=wa_in)
            nc.sync.dma_start(out=wam_t[:], in_=wam_in)
            nc.sync.dma_start(out=w1_t[:], in_=w1_in)
            nc.sync.dma_start(out=w2_t[:], in_=w2_in)
            for oi in range(2):
                nc.sync.dma_start(out=b1_t[:, oi : oi + 1], in_=b1_in[oi])
                nc.sync.dma_start(out=b2_t[:, oi : oi + 1], in_=b2_in[oi])
            nc.sync.dma_start(out=rm_t[:], in_=rm_in)
            nc.sync.dma_start(out=eye_t[:], in_=eye_in)

            p_t = ppool.tile([128, N, 2, RS, 64], F32)
            nc.sync.dma_start(out=p_t[:], in_=p_in)

            ones = cpool.tile([128, 1], F32)
            nc.vector.memset(ones[:], 1.0)
            ones_row = cpool.tile([1, 8], F32)
            nc.vector.memset(ones_row[:], 1.0)

            # ---- persistent feature tiles (border cols pre-zeroed) ----
            masked = [
                [
                    fpool.tile([128, 12, 66], F32R, name=f"mk{par}_{oi}", tag=f"mk{par}_{oi}")
                    for oi in range(2)
                ]
                for par in range(2)
            ]
            relu1 = [
                [
                    fpool.tile([128, 10, 66], F32R, name=f"r1{par}_{oi}", tag=f"r1{par}_{oi}")
                    for oi in range(2)
                ]
                for par in range(2)
            ]
            t_tiles = [
                [
                    fpool.tile([128, RS, 64], F32, name=f"t{n}_{oi}", tag=f"t{n}_{oi}")
                    for oi in range(2)
                ]
                for n in range(N)
            ]
            for par in range(2):
                for oi in range(2):
                    nc.sync.dma_start(out=masked[par][oi][:, :, 0:1], in_=zz_in[:, 0:12, 0:1])
                    nc.sync.dma_start(out=masked[par][oi][:, :, 65:66], in_=zz_in[:, 0:12, 1:2])
                    nc.sync.dma_start(out=relu1[par][oi][:, :, 0:1], in_=zz_in[:, 0:10, 0:1])
                    nc.sync.dma_start(out=relu1[par][oi][:, :, 65:66], in_=zz_in[:, 0:10, 1:2])

            # gram accumulators: acc[:, (i*8+n)*2+ci], tnorm[:, n*2+ci]
            acc = cpool.tile([128, 128], F32)
            tnorm = cpool.tile([128, 16], F32)

            # ---- per-image stages ----
            def conv1x1(n):
                xs_t = xpool.tile([128, 12, 64], F32R, name="xs_t", tag="xs_t")
                xsm_t = xpool.tile([1, 12, 64], F32R, name="xsm_t", tag="xsm_t")
                nc.sync.dma_start(out=xs_t[:], in_=xs_in[n])
                nc.sync.dma_start(out=xsm_t[:], in_=xsm_in[n])
                mk = masked[n % 2]
                for oi in range(2):
                    for r0, R in ((0, 8), (8, 4)):
                        ps = psum.tile([128, 512], F32, name="ps_c1", tag="ps")
                        ps3 = ps[:, 0 : R * 64].rearrange("p (r c) -> p r c", r=R)
                        nc.tensor.matmul(
                            ps3, wa_t[:, oi, :], xs_t[:, r0 : r0 + R, :],
                            start=True, stop=False,
                        )
                        nc.tensor.matmul(
                            ps3, wam_t[:, oi, :], xsm_t[:, r0 : r0 + R, :],
                            start=False, stop=True,
                        )
                        nc.scalar.copy(mk[oi][:, r0 : r0 + R, 1:65], ps3)

            def conv3x3(w_t, src, b_t, n, rows_out, fb, dst_fn, func):
                # dst_fn(oi) -> (out_ap rows x 64). src: 2-chunk padded tiles.
                for oi in range(2):
                    for r0, R in fb:
                        ps = psum.tile([128, 512], F32, name="ps_g", tag="ps")
                        ps3 = ps[:, 0 : R * 64].rearrange("p (r c) -> p r c", r=R)
                        k = 0
                        for t in range(9):
                            kh, kw = t // 3, t % 3
                            for ci in range(2):
                                nc.tensor.matmul(
                                    ps3,
                                    w_t[:, t, ci, oi, :],
                                    src[ci][:, r0 + kh : r0 + kh + R, kw : kw + 64],
                                    start=(k == 0),
                                    stop=(k == 17),
                                )
                                k += 1
                        nc.scalar.activation(
                            dst_fn(oi, r0, R), ps3, func, bias=b_t[:, oi : oi + 1],
                        )

            def gen1(n):
                mk, r1 = masked[n % 2], relu1[n % 2]
                conv3x3(
                    w1_t, mk, b1_t, n, 10, ((0, 5), (5, 5)),
                    lambda oi, r0, R: r1[oi][:, r0 : r0 + R, 1:65],
                    AF.Relu,
                )
                # zero out-of-image halo rows (rows 0 and 9) via rowmask
                for oi in range(2):
                    hal = r1[oi][:, 0:10:9, :]
                    nc.vector.tensor_tensor(
                        out=hal, in0=hal,
                        in1=rm_t[:, :, None].to_broadcast([128, 2, 66]),
                        op=ALU.mult,
                    )

            def gen2(n):
                r1 = relu1[n % 2]
                conv3x3(
                    w2_t, r1, b2_t, n, RS, ((0, 4), (4, 4)),
                    lambda oi, r0, R: t_tiles[n][oi][:, r0 : r0 + R, :],
                    AF.Identity,
                )

            def gram(n):
                for ci in range(2):
                    tt = t_tiles[n][ci]
                    sq = spool.tile([128, 512], F32, name="sq_s", tag="sq_s")
                    nc.scalar.activation(
                        sq[:].rearrange("p (r c) -> p r c", r=RS), tt[:], AF.Square,
                        accum_out=tnorm[:, n * 2 + ci : n * 2 + ci + 1],
                    )
                    for i in range(N):
                        gs = spool.tile([128, 512], F32, name="gr_s", tag="gr_s")
                        col = (i * 8 + n) * 2 + ci
                        nc.vector.affine_mul_reduce(
                            out=gs[:].rearrange("p (r c) -> p r c", r=RS),
                            accum_out=acc[:, col : col + 1],
                            in0=p_t[:, i, ci],
                            in1=tt[:],
                            scale=1.0,
                            bias=0.0,
                        )

            conv1x1(0)
            conv1x1(1)
            for n in range(N):
                gen1(n)
                gen2(n)
                if n + 2 < N:
                    conv1x1(n + 2)
                gram(n)

            # ---- CE tail (replicated on every core) ----
            acc64 = cpool.tile([128, 64], F32)
            accv = acc[:].rearrange("p (q c) -> p q c", c=2)
            nc.vector.tensor_tensor(out=acc64[:], in0=accv[:, :, 0], in1=accv[:, :, 1], op=ALU.add)
            tn8 = cpool.tile([128, 8], F32)
            tnv = tnorm[:].rearrange("p (q c) -> p q c", c=2)
            nc.vector.tensor_tensor(out=tn8[:], in0=tnv[:, :, 0], in1=tnv[:, :, 1], op=ALU.add)

            part = cps.tile([1, 72], F32, name="part", tag="ceps")
            nc.tensor.matmul(part[:, 0:64], ones[:], acc64[:], start=True, stop=False)
            nc.tensor.matmul(part[:, 64:72], ones[:], tn8[:], start=False, stop=True)
            part_sb = cpool.tile([1, 72], F32)
            nc.scalar.copy(part_sb[:], part[:])

            cc_in = dram.tile([1, 72], F32)
            cc_out = dram.tile([1, 72], F32)
            nc.sync.dma_start(out=cc_in[:], in_=part_sb[:])
            nc.gpsimd.collective_compute(
                "AllReduce", ALU.add, replica_groups=[list(range(N_CORES))],
                ins=[cc_in[:].opt()], outs=[cc_out[:].opt()],
            )

            q_sb = cpool.tile([8, 8], F32)
            qd = cpool.tile([8, 1], F32)
            tn_row = cpool.tile([1, 8], F32)
            tn_p = cpool.tile([8, 1], F32)
            cc_flat = cc_out[:].rearrange("a b -> (a b)")
            nc.sync.dma_start(
                out=q_sb[:], in_=cc_out[:, 0:64].rearrange("a (i j) -> (a i) j", i=8)
            )
            nc.sync.dma_start(out=qd[:], in_=cc_flat[0:64][::9].rearrange("(i j) -> i j", j=1))
            nc.sync.dma_start(out=tn_row[:], in_=cc_out[:, 64:72])
            nc.sync.dma_start(out=tn_p[:], in_=cc_flat[64:72].rearrange("(i j) -> i j", j=1))

            # L[i,j] = q/64 - tn[j]/128 via PE broadcast
            q_s = cpool.tile([8, 8], F32)
            nc.scalar.mul(q_s[:], q_sb[:], 1.0 / NOISE_VAR)
            tn_neg = cpool.tile([1, 8], F32)
            nc.scalar.mul(tn_neg[:], tn_row[:], -1.0 / (2.0 * NOISE_VAR))
            L_ps = cps.tile([8, 8], F32, name="L_ps", tag="ceps")
            nc.tensor.matmul(L_ps[:], eye_t[:], q_s[:], start=True, stop=False)
            nc.tensor.matmul(L_ps[:], ones_row[:], tn_neg[:], start=False, stop=True)
            L = cpool.tile([8, 8], F32)
            nc.vector.tensor_copy(L[:], L_ps[:])

            m = cpool.tile([8, 1], F32)
            nc.vector.reduce_max(m[:], L[:], axis=mybir.AxisListType.X)
            negm = cpool.tile([8, 1], F32)
            nc.scalar.mul(negm[:], m[:], -1.0)
            e = cpool.tile([8, 8], F32)
            nc.scalar.activation(e[:], L[:], AF.Exp, bias=negm[:, 0:1], scale=1.0)
            s = cpool.tile([8, 1], F32)
            nc.vector.reduce_sum(s[:], e[:], axis=mybir.AxisListType.X)
            ln_s = cpool.tile([8, 1], F32)
            nc.scalar.activation(ln_s[:], s[:], AF.Ln)

            ce = cpool.tile([8, 1], F32)
            nc.vector.tensor_add(ce[:], m[:], ln_s[:])
            t1 = cpool.tile([8, 1], F32)
            nc.scalar.mul(t1[:], qd[:], 1.0 / NOISE_VAR)
            nc.vector.tensor_sub(ce[:], ce[:], t1[:])
            t2 = cpool.tile([8, 1], F32)
            nc.scalar.mul(t2[:], tn_p[:], 1.0 / (2.0 * NOISE_VAR))
            nc.vector.tensor_add(ce[:], ce[:], t2[:])

            lp = cps.tile([1, 1], F32, name="lp", tag="ceps")
            nc.tensor.matmul(lp[:], ce[:], ones[0:8, :], start=True, stop=True)
            l_sb = cpool.tile([1, 1], F32)
            # loss = sum_i ce_i / N * (2*NOISE_VAR/N) * ALPHA
            nc.scalar.mul(l_sb[:], lp[:], (2.0 * NOISE_VAR / (N * N)) * ALPHA_MGD)
            nc.sync.dma_start(out=loss_out, in_=l_sb[:])
    nc.compile()
    return nc


def _prep_inputs(preds_S, preds_T, W_align, b_align, W_gen1, b_gen1, W_gen2, b_gen2):
    f32 = np.float32
    mat = ((np.arange(H)[:, None] + np.arange(W)[None, :]) % 2).astype(f32)

    wa = np.ascontiguousarray(W_align[:, :, 0, 0].T.reshape(128, 2, 128), f32)
    wam = np.ascontiguousarray(b_align.reshape(1, 2, 128), f32)

    def pack_w(Wg):
        w = Wg.reshape(2, 128, 2, 128, 3, 3)  # [oi, o, ci, i, kh, kw]
        w = w.transpose(3, 4, 5, 2, 0, 1)  # [i, kh, kw, ci, oi, o]
        return np.ascontiguousarray(w.reshape(128, 9, 2, 2, 128), f32)

    w1 = pack_w(np.asarray(W_gen1, f32))
    w2 = pack_w(np.asarray(W_gen2, f32))
    b1 = np.ascontiguousarray(b_gen1.reshape(2, 128, 1), f32)
    b2 = np.ascontiguousarray(b_gen2.reshape(2, 128, 1), f32)
    eye8 = np.eye(8, dtype=f32)
    zz = np.zeros((128, 12, 2), f32)

    in_maps = []
    for c in range(N_CORES):
        rows = np.arange(8 * c - 2, 8 * c + 10)
        valid = (rows >= 0) & (rows < H)
        vr = rows[valid]
        xs = np.zeros((N, 128, 12, 64), f32)
        xs[:, :, valid] = preds_S[:, :, vr, :] * mat[vr][None, None]
        xsm = np.zeros((N, 1, 12, 64), f32)
        xsm[:, 0, valid] = mat[vr]
        rm = np.broadcast_to(
            np.array([1.0 if c > 0 else 0.0, 1.0 if c < 7 else 0.0], f32), (128, 2)
        ).copy()
        slab = preds_T[:, :, 8 * c : 8 * c + RS, :].reshape(N, 2, 128, RS, 64)
        p = np.ascontiguousarray(slab.transpose(2, 0, 1, 3, 4), f32)
        in_maps.append(
            {
                "xs": xs, "xsm": xsm, "wa": wa, "wam": wam, "w1": w1, "w2": w2,
                "b1": b1, "b2": b2, "rm": rm, "p": p, "zz": zz, "eye8": eye8,
            }
        )
    return in_maps


def _make_runner(nc, n_cores):
    """Build a cached jitted SPMD runner (same mechanics as
    bass2jax.run_bass_via_pjrt, but reusable across calls)."""
    import jax
    from jax.experimental.shard_map import shard_map
    from jax.sharding import Mesh, PartitionSpec
    from concourse import bass2jax

    bass2jax.install_neuronx_cc_hook()
    assert nc.dbg_addr is None
    partition_name = nc.partition_id_tensor.name if nc.partition_id_tensor else None

    in_names, out_names, out_avals = [], [], []
    for alloc in nc.m.functions[0].allocations:
        if not isinstance(alloc, mybir.MemoryLocationSet):
            continue
        name = alloc.memorylocations[0].name
        if alloc.kind == "ExternalInput":
            if name != partition_name:
                in_names.append(name)
        elif alloc.kind == "ExternalOutput":
            out_names.append(name)
            out_avals.append(
                jax.core.ShapedArray(tuple(alloc.tensor_shape), mybir.dt.np(alloc.dtype))
            )
    n_params = len(in_names)
    n_outs = len(out_avals)
    all_names = tuple(in_names + out_names)
    if partition_name is not None:
        all_names = all_names + (partition_name,)
    donate = tuple(range(n_params, n_params + n_outs))

    def _body(*args):
        operands = list(args)
        if partition_name is not None:
            operands.append(bass2jax.partition_id_tensor())
        outs = bass2jax._bass_exec_p.bind(
            *operands,
            out_avals=tuple(out_avals),
            in_names=all_names,
            out_names=tuple(out_names),
            lowering_input_output_aliases=(),
            sim_require_finite=True,
            sim_require_nnan=True,
            nc=nc,
        )
        return tuple(outs)

    devices = jax.devices()[:n_cores]
    mesh = Mesh(np.asarray(devices), ("core",))
    in_specs = (PartitionSpec("core"),) * (n_params + n_outs)
    out_specs = (PartitionSpec("core"),) * n_outs
    sharded = jax.jit(
        shard_map(_body, mesh=mesh, in_specs=in_specs, out_specs=out_specs, check_rep=False),
        donate_argnums=donate,
        keep_unused=True,
    )

    def run(in_maps):
        concat_in = [
            np.concatenate([np.asarray(in_maps[c][k]) for c in range(n_cores)], axis=0)
            for k in in_names
        ]
        concat_zeros = [
            np.zeros((n_cores * a.shape[0], *a.shape[1:]), a.dtype) for a in out_avals
        ]
        out_arrs = sharded(*concat_in, *concat_zeros)
        return [
            {
                k: np.asarray(out_arrs[i]).reshape(n_cores, *out_avals[i].shape)[c]
                for i, k in enumerate(out_names)
            }
            for c in range(n_cores)
        ]

    return run


def kernel(preds_S, preds_T, W_align, b_align, W_gen1, b_gen1, W_gen2, b_gen2):
    global LAST_RESULTS
    preds_S = np.asarray(preds_S, np.float32)
    preds_T = np.asarray(preds_T, np.float32)
    in_maps = _prep_inputs(
        preds_S, preds_T,
        np.asarray(W_align, np.float32), np.asarray(b_align, np.float32),
        np.asarray(W_gen1, np.float32), np.asarray(b_gen1, np.float32),
        np.asarray(W_gen2, np.float32), np.asarray(b_gen2, np.float32),
    )
    if "run" not in _NC_CACHE:
        _NC_CACHE["run"] = _make_runner(_build(), N_CORES)
    results = _NC_CACHE["run"](in_maps)
    LAST_RESULTS = results
    return np.float32(results[0]["loss"][0, 0])
